# revision 4
# baseline (speedup 1.0000x reference)
"""Trainium2 Bass kernel: 16-head MHA (B=2, S=2048, E=1024) on 8 NeuronCores.

Sharding: core c = (batch b = c // 4, head-group g = c % 4); each core runs
4 heads of one batch (data parallel on B x tensor parallel on heads).  The
output projection is row-sharded: each core produces a partial [S, E] f32
output; the host sums the 4 head-group partials per batch and adds bo.

Device pipeline per core (all matmul operands bf16, fp32 PSUM accumulation):
  qT[d,m] = WqT.T-contract-e(xqT)     (weight-tile stationary, reused over
                                       all 4 m-chunks before switching)
  kT[d,n] = same
  v[n,dv] = xvT-tile stationary split into two 64-row groups (concurrent
            matmul pairs), WvT moving; merged+cast into v_aug with a ones
            column per head so the attention matmul also yields softmax
            denominators
  scoresT[n,m] = kT-tile stationary (K=64; the two heads of a pair run in
            different PE row groups concurrently), qT moving
  probsT = exp(scoresT / sqrt(dk)) via one ACT per (j, head-pair); causal
            masking by skipping fully-masked n-tiles plus gpsimd memset /
            0-1 multiply on diagonal tiles
  o_aug[dv+1,m] = v_aug stationary, probsT moving, accumulated over n-tiles,
            software-pipelined one step behind the scores matmuls; row 64
            is sum(probs) = softmax denominator
  oT = o_aug[0:64] * broadcast(1/o_aug[64])   (reciprocal_approx_fast +
            gpsimd partition_broadcast)
  out[m,e] partial = oT-tile stationary (serves both e-chunks), WoT moving
"""

import numpy as np
import ml_dtypes

B, S, E = 2, 2048, 1024
H, DK = 16, 64
NCORES = 8
G = 4                 # head-groups (tensor parallel degree)
NH = H // G           # heads per core = 4
DKH = NH * DK         # 256 head dims per core
P = 128
MC = 512              # m-chunk (psum bank width in f32)
NMC = S // MC         # 4 m-chunks
NT = S // P           # 16 n-tiles (and m-tiles)
ET = E // P           # 8 e-tiles
PAIRS = NH // 2       # 2 head pairs per core
BF16 = ml_dtypes.bfloat16
SCALE = float(1.0 / np.sqrt(np.float32(DK)))


def _build_program(chunk_ntiles, causal, bias_qk, bias_v):
    """Build the (SPMD, shared across all 8 cores) Bass program.

    chunk_ntiles[c] = number of 128-wide n-tiles to process for m-chunk c.
    causal: apply diagonal-tile masking (memset + tri multiply).
    """
    from contextlib import ExitStack

    import concourse.bass as bass
    import concourse.tile as tile
    from concourse import bacc, mybir

    f32 = mybir.dt.float32
    bf16 = mybir.dt.bfloat16
    Exp = mybir.ActivationFunctionType.Exp

    nc = bacc.Bacc(
        "TRN2",
        target_bir_lowering=False,
        debug=False,
        enable_asserts=False,
        num_devices=NCORES,
    )

    # ---- DRAM I/O ----
    xqT = nc.dram_tensor("xqT", [E, S], bf16, kind="ExternalInput").ap()
    xkT = nc.dram_tensor("xkT", [E, S], bf16, kind="ExternalInput").ap()
    xvT = nc.dram_tensor("xvT", [E, S], bf16, kind="ExternalInput").ap()
    wqT = nc.dram_tensor("wqT", [E, DKH], bf16, kind="ExternalInput").ap()
    wkT = nc.dram_tensor("wkT", [E, DKH], bf16, kind="ExternalInput").ap()
    wvT = nc.dram_tensor("wvT", [E, DKH], bf16, kind="ExternalInput").ap()
    woT = nc.dram_tensor("woT", [DKH, E], bf16, kind="ExternalInput").ap()
    dmask = nc.dram_tensor("dmask", [P, P], bf16, kind="ExternalInput").ap()
    if bias_qk:
        bqd = nc.dram_tensor("bq", [DKH, 1], f32, kind="ExternalInput").ap()
        bkd = nc.dram_tensor("bk", [DKH, 1], f32, kind="ExternalInput").ap()
    if bias_v:
        bvd = nc.dram_tensor("bv", [1, DKH], f32, kind="ExternalInput").ap()
    out = nc.dram_tensor("out", [S, E], f32, kind="ExternalOutput").ap()

    with tile.TileContext(nc) as tc, ExitStack() as ctx:
        const = ctx.enter_context(tc.tile_pool(name="const", bufs=1))
        xpool = ctx.enter_context(tc.tile_pool(name="xpool", bufs=1))
        wpool = ctx.enter_context(tc.tile_pool(name="wpool", bufs=1))
        qkpool = ctx.enter_context(tc.tile_pool(name="qkpool", bufs=1))
        vpool = ctx.enter_context(tc.tile_pool(name="vpool", bufs=1))
        prpool = ctx.enter_context(tc.tile_pool(name="prpool", bufs=8))
        rcpool = ctx.enter_context(tc.tile_pool(name="rcpool", bufs=4))
        otpool = ctx.enter_context(tc.tile_pool(name="otpool", bufs=1))
        ostpool = ctx.enter_context(tc.tile_pool(name="ostpool", bufs=4))

        # ---- constants / weights first (small, unblock first matmuls) ----
        dmask_sb = const.tile([P, P], bf16, tag="dmask")
        nc.sync.dma_start(out=dmask_sb, in_=dmask)

        # weight tiles: [e-within-tile, e-tile, d]
        wk_sb = wpool.tile([P, ET, DKH], bf16, tag="wk")
        nc.sync.dma_start(out=wk_sb, in_=wkT.rearrange("(t p) d -> p t d", p=P))
        wq_sb = wpool.tile([P, ET, DKH], bf16, tag="wq")
        nc.sync.dma_start(out=wq_sb, in_=wqT.rearrange("(t p) d -> p t d", p=P))
        wv_sb = wpool.tile([P, ET, DKH], bf16, tag="wv")
        nc.sync.dma_start(out=wv_sb, in_=wvT.rearrange("(t p) d -> p t d", p=P))
        wo_sb = wpool.tile([P, PAIRS, E], bf16, tag="wo")
        nc.sync.dma_start(out=wo_sb, in_=woT.rearrange("(t p) e -> p t e", p=P))

        if bias_qk:
            bq_sb = const.tile([P, PAIRS], f32, tag="bq")
            nc.sync.dma_start(out=bq_sb, in_=bqd.rearrange("(t p) o -> p (t o)", p=P))
            bk_sb = const.tile([P, PAIRS], f32, tag="bk")
            nc.sync.dma_start(out=bk_sb, in_=bkd.rearrange("(t p) o -> p (t o)", p=P))
        if bias_v:
            bv_row = const.tile([1, DKH], f32, tag="bv_row")
            nc.sync.dma_start(out=bv_row, in_=bvd)
            bv_sb = const.tile([P, DKH], f32, tag="bv_bc")
            nc.gpsimd.partition_broadcast(bv_sb, bv_row)

        # x inputs, chunked DMAs so the first projections start early
        xk_sb, xq_sb, xv_sb = [], [], []
        for i in range(ET):
            t = xpool.tile([P, S], bf16, tag=f"xk{i}", name=f"xk_sb{i}")
            xk_sb.append(t)
            t = xpool.tile([P, S], bf16, tag=f"xq{i}", name=f"xq_sb{i}")
            xq_sb.append(t)
            t = xpool.tile([P, S], bf16, tag=f"xv{i}", name=f"xv_sb{i}")
            xv_sb.append(t)
        for c in range(NMC):
            cs = slice(MC * c, MC * (c + 1))
            for i in range(ET):
                nc.sync.dma_start(out=xk_sb[i][:, cs], in_=xkT[P * i : P * (i + 1), cs])
                nc.sync.dma_start(out=xq_sb[i][:, cs], in_=xqT[P * i : P * (i + 1), cs])
                nc.sync.dma_start(out=xv_sb[i][:, cs], in_=xvT[P * i : P * (i + 1), cs])

        # persistent activation tiles
        qT_sb = [qkpool.tile([P, S], bf16, tag=f"qT{p}", name=f"qT_sb{p}") for p in range(PAIRS)]
        kT_sb = [qkpool.tile([P, S], bf16, tag=f"kT{p}", name=f"kT_sb{p}") for p in range(PAIRS)]
        vaug_sb = [vpool.tile([P, NH, DK + 1], bf16, tag=f"va{j}", name=f"vaug_sb{j}") for j in range(NT)]
        oT_sb = [otpool.tile([P, S], bf16, tag=f"oT{p}", name=f"oT_sb{p}") for p in range(PAIRS)]

        # ---- stage 1: projections ----
        # q/k: weight-tile stationary, streamed over all 4 chunks (psum x4)
        # v:   x-tile stationary split into two row-group halves (concurrent)
        with tc.tile_pool(name="pj_ps", bufs=4, space="PSUM") as pjps:
            for p in range(PAIRS):
                for dst, w_sb, x_sb, bias in (
                    (kT_sb, wk_sb, xk_sb, bk_sb if bias_qk else None),
                    (qT_sb, wq_sb, xq_sb, bq_sb if bias_qk else None),
                ):
                    ps = [pjps.tile([P, MC], f32, tag="qk", name="ps_qk") for _ in range(NMC)]
                    for i in range(ET):
                        for c in range(NMC):
                            nc.tensor.matmul(
                                ps[c],
                                w_sb[:, i, P * p : P * (p + 1)],
                                x_sb[i][:, MC * c : MC * (c + 1)],
                                start=(i == 0),
                                stop=(i == ET - 1),
                            )
                    for c in range(NMC):
                        dslice = dst[p][:, MC * c : MC * (c + 1)]
                        if bias is not None:
                            nc.vector.tensor_scalar_add(dslice, ps[c], bias[:, p : p + 1])
                        else:
                            nc.scalar.copy(dslice, ps[c])
            for j in range(NT):
                psA = pjps.tile([P, DKH], f32, tag="vA", name="ps_vA", bufs=2)
                psB = pjps.tile([P, DKH], f32, tag="vB", name="ps_vB", bufs=2)
                for i in range(ET):
                    nc.tensor.matmul(
                        psA,
                        xv_sb[i][0:64, P * j : P * (j + 1)],
                        wv_sb[0:64, i, :],
                        start=(i == 0),
                        stop=(i == ET - 1),
                    )
                    nc.tensor.matmul(
                        psB,
                        xv_sb[i][64:P, P * j : P * (j + 1)],
                        wv_sb[64:P, i, :],
                        start=(i == 0),
                        stop=(i == ET - 1),
                    )
                va = vaug_sb[j]
                # DVE can read only one PSUM operand per op: stage B via SBUF
                vtmp = rcpool.tile([P, DKH], f32, tag="vtmp", name="vtmp_t", bufs=2)
                nc.scalar.copy(vtmp, psB)
                pA3 = psA.rearrange("p (h d) -> p h d", h=NH)
                tB3 = vtmp.rearrange("p (h d) -> p h d", h=NH)
                nc.vector.tensor_add(va[:, :, 0:DK], pA3, tB3)
                if bias_v:
                    bv3 = bv_sb.rearrange("p (h d) -> p h d", h=NH)
                    nc.vector.tensor_add(va[:, :, 0:DK], va[:, :, 0:DK], bv3)
                nc.vector.memset(va[:, :, DK : DK + 1], 1.0)

        # ---- stage 2: attention ----
        with (
            tc.tile_pool(name="sc_ps", bufs=2, space="PSUM") as scps,
            tc.tile_pool(name="oa_ps", bufs=4, space="PSUM") as oaps,
        ):
            for c in range(NMC):
                J = chunk_ntiles[c]
                for p in range(PAIRS):
                    oaug = [
                        oaps.tile([P, MC], f32, tag="oaug", name=f"oaug{h01}")
                        for h01 in range(2)
                    ]
                    probs_tiles = [None] * J

                    def scores_step(j):
                        sc = scps.tile([P, 2 * MC], f32, tag="sc", name="sc_ps_t")
                        for h01 in range(2):
                            nc.tensor.matmul(
                                sc[:, MC * h01 : MC * (h01 + 1)],
                                kT_sb[p][64 * h01 : 64 * (h01 + 1), P * j : P * (j + 1)],
                                qT_sb[p][64 * h01 : 64 * (h01 + 1), MC * c : MC * (c + 1)],
                                start=True,
                                stop=True,
                            )
                        probs = prpool.tile([P, 2 * MC], bf16, tag="probs", name="probs_t")
                        nc.scalar.activation(probs, sc, Exp, bias=0.0, scale=SCALE)
                        if causal and j >= 4 * c:
                            off = P * (j - 4 * c)
                            for h01 in range(2):
                                base = MC * h01
                                if off > 0:
                                    nc.gpsimd.memset(probs[:, base : base + off], 0.0)
                                nc.gpsimd.tensor_mul(
                                    probs[:, base + off : base + off + P],
                                    probs[:, base + off : base + off + P],
                                    dmask_sb,
                                )
                        probs_tiles[j] = probs

                    def attnv_step(j):
                        probs = probs_tiles[j]
                        for h01 in range(2):
                            h = 2 * p + h01
                            nc.tensor.matmul(
                                oaug[h01][0 : DK + 1, :],
                                vaug_sb[j][:, h, :],
                                probs[:, MC * h01 : MC * (h01 + 1)],
                                start=(j == 0),
                                stop=(j == J - 1),
                            )

                    # software pipeline: scores one step ahead of attnV
                    for j in range(J):
                        scores_step(j)
                        if j >= 1:
                            attnv_step(j - 1)
                    attnv_step(J - 1)

                    for h01 in range(2):
                        rc = rcpool.tile([1, MC], f32, tag="rc", name="rc_t")
                        nc.vector.reciprocal(rc, oaug[h01][DK : DK + 1, :])
                        bc = rcpool.tile([64, MC], f32, tag="bc", name="bc_t")
                        nc.gpsimd.partition_broadcast(bc, rc)
                        nc.vector.tensor_mul(
                            oT_sb[p][64 * h01 : 64 * (h01 + 1), MC * c : MC * (c + 1)],
                            oaug[h01][0:DK, :],
                            bc,
                        )

        # ---- stage 3: output projection ----
        with tc.tile_pool(name="op_ps", bufs=4, space="PSUM") as opps:
            for t in range(NT):
                op = [
                    opps.tile([P, MC], f32, tag="op", name="op_t")
                    for _ in range(E // MC)
                ]
                for p in range(PAIRS):
                    for ec in range(E // MC):
                        nc.tensor.matmul(
                            op[ec],
                            oT_sb[p][:, P * t : P * (t + 1)],
                            wo_sb[:, p, MC * ec : MC * (ec + 1)],
                            start=(p == 0),
                            stop=(p == PAIRS - 1),
                        )
                for ec in range(E // MC):
                    ost = ostpool.tile([P, MC], f32, tag="ost", name="ost_t")
                    nc.vector.tensor_copy(ost, op[ec])
                    nc.sync.dma_start(
                        out=out[P * t : P * (t + 1), MC * ec : MC * (ec + 1)],
                        in_=ost,
                    )

    nc.compile()
    return nc


def _host_inputs(key, value, query, Wk, Wq, Wv, Wo, bq, bk, bv, bias_qk, bias_v):
    """Per-core input maps (host-side shard/transpose/cast — not timed)."""
    tri = np.triu(np.ones((P, P), np.float32)).astype(BF16)  # allowed: n<=m
    in_maps = []
    xT = {}
    for b in range(B):
        xT[("q", b)] = np.ascontiguousarray(query[b].T).astype(BF16)
        xT[("k", b)] = np.ascontiguousarray(key[b].T).astype(BF16)
        xT[("v", b)] = np.ascontiguousarray(value[b].T).astype(BF16)
    for c in range(NCORES):
        b, g = divmod(c, G)
        sl = slice(DKH * g, DKH * (g + 1))
        m = {
            "xqT": xT[("q", b)],
            "xkT": xT[("k", b)],
            "xvT": xT[("v", b)],
            "wqT": np.ascontiguousarray(Wq[sl].T).astype(BF16),
            "wkT": np.ascontiguousarray(Wk[sl].T).astype(BF16),
            "wvT": np.ascontiguousarray(Wv[sl].T).astype(BF16),
            "woT": np.ascontiguousarray(Wo[:, sl].T).astype(BF16),
            "dmask": tri,
        }
        if bias_qk:
            m["bq"] = np.ascontiguousarray(bq[sl].astype(np.float32).reshape(DKH, 1))
            m["bk"] = np.ascontiguousarray(bk[sl].astype(np.float32).reshape(DKH, 1))
        if bias_v:
            m["bv"] = np.ascontiguousarray(bv[sl].astype(np.float32).reshape(1, DKH))
        in_maps.append(m)
    return in_maps


def _numpy_fallback(key, value, query, mask, Wk, bk, Wq, bq, Wv, bv, Wo, bo):
    """Exact reference semantics in numpy (general-mask fallback)."""
    def proj(x, W, b):
        return x @ W.T + b

    k = proj(key, Wk, bk).reshape(B, S, H, DK).transpose(0, 2, 1, 3)
    q = proj(query, Wq, bq).reshape(B, S, H, DK).transpose(0, 2, 1, 3)
    v = proj(value, Wv, bv).reshape(B, S, H, DK).transpose(0, 2, 1, 3)
    scores = np.einsum("bhmd,bhnd->bhmn", q, k).astype(np.float32)
    scores = np.where(mask, scores, np.float32(-1e10)) * np.float32(SCALE)
    scores -= scores.max(axis=3, keepdims=True)
    e = np.exp(scores)
    attn = e / e.sum(axis=3, keepdims=True)
    o = np.einsum("bhmn,bhnv->bhmv", attn, v)
    o = o.transpose(0, 2, 1, 3).reshape(B, S, E)
    return (o @ Wo.T + bo).astype(np.float32)


_program_cache = {}


def kernel(key, value, query, mask, Wk, bk, Wq, bq, Wv, bv, Wo, bo):
    key = np.asarray(key, np.float32)
    value = np.asarray(value, np.float32)
    query = np.asarray(query, np.float32)
    mask = np.asarray(mask)
    Wk, bk = np.asarray(Wk, np.float32), np.asarray(bk, np.float32)
    Wq, bq = np.asarray(Wq, np.float32), np.asarray(bq, np.float32)
    Wv, bv = np.asarray(Wv, np.float32), np.asarray(bv, np.float32)
    Wo, bo = np.asarray(Wo, np.float32), np.asarray(bo, np.float32)

    m2 = mask.reshape(B, S, S) if mask.size == B * S * S else None
    causal = m2 is not None and all(
        np.array_equal(m2[b], np.tril(np.ones((S, S), bool))) for b in range(B)
    )
    allones = m2 is not None and bool(mask.all())
    if not causal and not allones:
        return _numpy_fallback(key, value, query, mask, Wk, bk, Wq, bq, Wv, bv, Wo, bo)

    if causal:
        chunk_ntiles = tuple(4 * (c + 1) for c in range(NMC))
    else:
        chunk_ntiles = tuple(NT for _ in range(NMC))

    bias_qk = bool(np.any(bq) or np.any(bk))
    bias_v = bool(np.any(bv))

    pkey = (chunk_ntiles, causal, bias_qk, bias_v)
    if pkey not in _program_cache:
        _program_cache[pkey] = _build_program(chunk_ntiles, causal, bias_qk, bias_v)
    nc = _program_cache[pkey]

    from concourse.bass_utils import run_bass_kernel_spmd

    in_maps = _host_inputs(key, value, query, Wk, Wq, Wv, Wo, bq, bk, bv, bias_qk, bias_v)
    res = run_bass_kernel_spmd(nc, in_maps, core_ids=list(range(NCORES)))

    outp = np.zeros((B, S, E), np.float32)
    for c in range(NCORES):
        outp[c // G] += res.results[c]["out"]
    outp += bo.astype(np.float32)
    return outp


# revision 9
# speedup vs baseline: 1.5030x; 1.5030x over previous
"""Trainium2 Bass kernel: 16-head MHA (B=2, S=2048, E=1024) on 8 NeuronCores.

Sharding: core c = (batch b = c // 4, head-group g = c % 4); each core runs
4 heads of one batch (data parallel on B x tensor parallel on heads).  The
output projection is row-sharded: each core produces a partial [S, E] f32
output; the host sums the 4 head-group partials per batch and adds bo.

Device pipeline per core (all matmul operands bf16, fp32 PSUM accumulation):
  qT[d,m] = WqT.T-contract-e(xqT)     (weight-tile stationary, reused over
                                       all 4 m-chunks before switching)
  kT[d,n] = same
  v[n,dv] = xvT-tile stationary split into two 64-row groups (concurrent
            matmul pairs), WvT moving; merged+cast into v_aug with a ones
            column per head so the attention matmul also yields softmax
            denominators
  scoresT[n,m] = kT-tile stationary (K=64; the two heads of a pair run in
            different PE row groups concurrently), qT moving
  probsT = exp(scoresT / sqrt(dk)) via one ACT per (j, head-pair); causal
            masking by skipping fully-masked n-tiles plus gpsimd memset /
            0-1 multiply on diagonal tiles
  o_aug[dv+1,m] = v_aug stationary, probsT moving, accumulated over n-tiles,
            software-pipelined one step behind the scores matmuls; row 64
            is sum(probs) = softmax denominator
  oT = o_aug[0:64] * broadcast(1/o_aug[64])   (reciprocal_approx_fast +
            gpsimd partition_broadcast)
  out[m,e] partial = oT-tile stationary (serves both e-chunks), WoT moving
"""

import numpy as np
import ml_dtypes

B, S, E = 2, 2048, 1024
H, DK = 16, 64
NCORES = 8
G = 4                 # head-groups (tensor parallel degree)
NH = H // G           # heads per core = 4
DKH = NH * DK         # 256 head dims per core
P = 128
MC = 512              # m-chunk (psum bank width in f32)
NMC = S // MC         # 4 m-chunks
NT = S // P           # 16 n-tiles (and m-tiles)
ET = E // P           # 8 e-tiles
PAIRS = NH // 2       # 2 head pairs per core
BF16 = ml_dtypes.bfloat16
SCALE = float(1.0 / np.sqrt(np.float32(DK)))


def _build_program(chunk_ntiles, causal, bias_qk, bias_v):
    """Build the (SPMD, shared across all 8 cores) Bass program.

    chunk_ntiles[c] = number of 128-wide n-tiles to process for m-chunk c.
    causal: apply diagonal-tile masking (memset + tri multiply).
    """
    from contextlib import ExitStack

    import concourse.bass as bass
    import concourse.tile as tile
    from concourse import bacc, mybir

    f32 = mybir.dt.float32
    bf16 = mybir.dt.bfloat16
    Exp = mybir.ActivationFunctionType.Exp

    nc = bacc.Bacc(
        "TRN2",
        target_bir_lowering=False,
        debug=False,
        enable_asserts=False,
        num_devices=NCORES,
    )

    # ---- DRAM I/O ----
    xqT = nc.dram_tensor("xqT", [E, S], bf16, kind="ExternalInput").ap()
    xkT = nc.dram_tensor("xkT", [E, S], bf16, kind="ExternalInput").ap()
    xvT = nc.dram_tensor("xvT", [E, S], bf16, kind="ExternalInput").ap()
    wqT = nc.dram_tensor("wqT", [E, DKH], bf16, kind="ExternalInput").ap()
    wkT = nc.dram_tensor("wkT", [E, DKH], bf16, kind="ExternalInput").ap()
    wvT = nc.dram_tensor("wvT", [E, DKH], bf16, kind="ExternalInput").ap()
    woT = nc.dram_tensor("woT", [DKH, E], bf16, kind="ExternalInput").ap()
    dmask = nc.dram_tensor("dmask", [P, P], bf16, kind="ExternalInput").ap()
    if bias_qk:
        bqd = nc.dram_tensor("bq", [DKH, 1], f32, kind="ExternalInput").ap()
        bkd = nc.dram_tensor("bk", [DKH, 1], f32, kind="ExternalInput").ap()
    if bias_v:
        bvd = nc.dram_tensor("bv", [1, DKH], f32, kind="ExternalInput").ap()
    out = nc.dram_tensor("out", [S, E], f32, kind="ExternalOutput").ap()

    with tile.TileContext(nc) as tc, ExitStack() as ctx:
        const = ctx.enter_context(tc.tile_pool(name="const", bufs=1))
        xpool = ctx.enter_context(tc.tile_pool(name="xpool", bufs=1))
        wpool = ctx.enter_context(tc.tile_pool(name="wpool", bufs=1))
        qkpool = ctx.enter_context(tc.tile_pool(name="qkpool", bufs=1))
        vpool = ctx.enter_context(tc.tile_pool(name="vpool", bufs=1))
        prpool = ctx.enter_context(tc.tile_pool(name="prpool", bufs=8))
        rcpool = ctx.enter_context(tc.tile_pool(name="rcpool", bufs=4))
        otpool = ctx.enter_context(tc.tile_pool(name="otpool", bufs=1))
        ostpool = ctx.enter_context(tc.tile_pool(name="ostpool", bufs=4))

        # ---- constants / weights first (small, unblock first matmuls) ----
        dmask_sb = const.tile([P, P], bf16, tag="dmask")
        nc.sync.dma_start(out=dmask_sb, in_=dmask)

        # weight tiles: [e-within-tile, e-tile, d]
        wk_sb = wpool.tile([P, ET, DKH], bf16, tag="wk")
        nc.sync.dma_start(out=wk_sb, in_=wkT.rearrange("(t p) d -> p t d", p=P))
        wq_sb = wpool.tile([P, ET, DKH], bf16, tag="wq")
        nc.sync.dma_start(out=wq_sb, in_=wqT.rearrange("(t p) d -> p t d", p=P))
        wv_sb = wpool.tile([P, ET, DKH], bf16, tag="wv")
        nc.sync.dma_start(out=wv_sb, in_=wvT.rearrange("(t p) d -> p t d", p=P))
        wo_sb = wpool.tile([P, PAIRS, E], bf16, tag="wo")
        nc.sync.dma_start(out=wo_sb, in_=woT.rearrange("(t p) e -> p t e", p=P))

        if bias_qk:
            bq_sb = const.tile([P, PAIRS], f32, tag="bq")
            nc.sync.dma_start(out=bq_sb, in_=bqd.rearrange("(t p) o -> p (t o)", p=P))
            bk_sb = const.tile([P, PAIRS], f32, tag="bk")
            nc.sync.dma_start(out=bk_sb, in_=bkd.rearrange("(t p) o -> p (t o)", p=P))
        if bias_v:
            bv_row = const.tile([1, DKH], f32, tag="bv_row")
            nc.sync.dma_start(out=bv_row, in_=bvd)
            bv_sb = const.tile([P, DKH], f32, tag="bv_bc")
            nc.gpsimd.partition_broadcast(bv_sb, bv_row)

        # x inputs, chunked DMAs so the first projections start early
        xk_sb, xq_sb, xv_sb = [], [], []
        for i in range(ET):
            t = xpool.tile([P, S], bf16, tag=f"xk{i}", name=f"xk_sb{i}")
            xk_sb.append(t)
            t = xpool.tile([P, S], bf16, tag=f"xq{i}", name=f"xq_sb{i}")
            xq_sb.append(t)
            t = xpool.tile([P, S], bf16, tag=f"xv{i}", name=f"xv_sb{i}")
            xv_sb.append(t)
        for c in range(NMC):
            cs = slice(MC * c, MC * (c + 1))
            for i in range(ET):
                nc.sync.dma_start(out=xk_sb[i][:, cs], in_=xkT[P * i : P * (i + 1), cs])
                nc.sync.dma_start(out=xq_sb[i][:, cs], in_=xqT[P * i : P * (i + 1), cs])
                nc.sync.dma_start(out=xv_sb[i][:, cs], in_=xvT[P * i : P * (i + 1), cs])

        # persistent activation tiles
        qT_sb = [qkpool.tile([P, S], bf16, tag=f"qT{p}", name=f"qT_sb{p}") for p in range(PAIRS)]
        kT_sb = [qkpool.tile([P, S], bf16, tag=f"kT{p}", name=f"kT_sb{p}") for p in range(PAIRS)]
        vaug_sb = [vpool.tile([P, NH, DK + 1], bf16, tag=f"va{j}", name=f"vaug_sb{j}") for j in range(NT)]
        oT_sb = [otpool.tile([P, S], bf16, tag=f"oT{p}", name=f"oT_sb{p}") for p in range(PAIRS)]

        # ---- stage 1: projections ----
        # q/k: weight-tile stationary, streamed over all 4 chunks (psum x4)
        # v:   x-tile stationary split into two row-group halves (concurrent)
        with tc.tile_pool(name="pj_ps", bufs=4, space="PSUM") as pjps:
            for p in range(PAIRS):
                for dst, w_sb, x_sb, bias in (
                    (kT_sb, wk_sb, xk_sb, bk_sb if bias_qk else None),
                    (qT_sb, wq_sb, xq_sb, bq_sb if bias_qk else None),
                ):
                    ps = [pjps.tile([P, MC], f32, tag="qk", name="ps_qk") for _ in range(NMC)]
                    for i in range(ET):
                        for c in range(NMC):
                            nc.tensor.matmul(
                                ps[c],
                                w_sb[:, i, P * p : P * (p + 1)],
                                x_sb[i][:, MC * c : MC * (c + 1)],
                                start=(i == 0),
                                stop=(i == ET - 1),
                            )
                    for c in range(NMC):
                        dslice = dst[p][:, MC * c : MC * (c + 1)]
                        if bias is not None:
                            nc.vector.tensor_scalar_add(dslice, ps[c], bias[:, p : p + 1])
                        else:
                            nc.scalar.copy(dslice, ps[c])
            for j in range(NT):
                psv = pjps.tile([P, DKH], f32, tag="v", name="ps_v", bufs=2)
                for i in range(ET):
                    nc.tensor.matmul(
                        psv,
                        xv_sb[i][:, P * j : P * (j + 1)],
                        wv_sb[:, i, :],
                        start=(i == 0),
                        stop=(i == ET - 1),
                    )
                va = vaug_sb[j]
                psv3 = psv.rearrange("p (h d) -> p h d", h=NH)
                if bias_v:
                    bv3 = bv_sb.rearrange("p (h d) -> p h d", h=NH)
                    nc.vector.tensor_add(va[:, :, 0:DK], psv3, bv3)
                else:
                    nc.vector.tensor_copy(va[:, :, 0:DK], psv3)
                nc.vector.memset(va[:, :, DK : DK + 1], 1.0)

        # ---- stage 2: attention ----
        with (
            tc.tile_pool(name="sc_ps", bufs=2, space="PSUM") as scps,
            tc.tile_pool(name="oa_ps", bufs=4, space="PSUM") as oaps,
            tc.tile_pool(name="rc_dram", bufs=4, space="DRAM") as rcdram,
        ):
            for c in range(NMC):
                J = chunk_ntiles[c]
                for p in range(PAIRS):
                    oaug = [
                        oaps.tile([P, MC], f32, tag="oaug", name=f"oaug{h01}")
                        for h01 in range(2)
                    ]
                    probs_tiles = [None] * J

                    def scores_step(j):
                        # columns left of `off` in this m-chunk are fully
                        # masked for n-tile j: never compute/exp/consume them
                        off = P * (j - 4 * c) if (causal and j >= 4 * c) else 0
                        w = MC - off
                        sc = scps.tile([P, 2 * MC], f32, tag="sc", name="sc_ps_t")
                        for h01 in range(2):
                            nc.tensor.matmul(
                                sc[:, MC * h01 + off : MC * (h01 + 1)],
                                kT_sb[p][64 * h01 : 64 * (h01 + 1), P * j : P * (j + 1)],
                                qT_sb[p][64 * h01 : 64 * (h01 + 1), MC * c + off : MC * (c + 1)],
                                start=True,
                                stop=True,
                            )
                        probs = prpool.tile([P, 2 * MC], bf16, tag="probs", name="probs_t")
                        sc3 = sc.rearrange("p (u m) -> p u m", u=2)
                        pr3 = probs.rearrange("p (u m) -> p u m", u=2)
                        nc.scalar.activation(
                            pr3[:, :, off:MC], sc3[:, :, off:MC], Exp, bias=0.0, scale=SCALE
                        )
                        if causal and j >= 4 * c:
                            for h01 in range(2):
                                base = MC * h01 + off
                                nc.gpsimd.tensor_mul(
                                    probs[:, base : base + P],
                                    probs[:, base : base + P],
                                    dmask_sb,
                                )
                        probs_tiles[j] = (probs, off)

                    def attnv_step(j):
                        probs, off = probs_tiles[j]
                        for h01 in range(2):
                            h = 2 * p + h01
                            nc.tensor.matmul(
                                oaug[h01][0 : DK + 1, off:MC],
                                vaug_sb[j][:, h, :],
                                probs[:, MC * h01 + off : MC * (h01 + 1)],
                                start=(j == 0),
                                stop=(j == J - 1),
                            )

                    # software pipeline: scores one step ahead of attnV
                    for j in range(J):
                        scores_step(j)
                        if j >= 1:
                            attnv_step(j - 1)
                    attnv_step(J - 1)

                    for h01 in range(2):
                        rc = rcpool.tile([1, MC], f32, tag="rc", name="rc_t")
                        nc.vector.reciprocal(rc, oaug[h01][DK : DK + 1, :])
                        bc = rcpool.tile([64, MC], f32, tag="bc", name="bc_t")
                        # replicate across partitions: bounce through DRAM,
                        # re-read with a step-0 partition AP (legal for DRAM
                        # sources) — keeps the broadcast off all compute queues
                        rcd = rcdram.tile([1, MC], f32, tag="rcd", name="rcd_t")
                        nc.sync.dma_start(out=rcd, in_=rc)
                        rc_bcast = bass.AP(
                            tensor=rcd.tensor,
                            offset=rcd.offset,
                            ap=[[0, 64]] + [list(a) for a in rcd.ap[1:]],
                        )
                        nc.sync.dma_start(out=bc, in_=rc_bcast)
                        nc.vector.tensor_mul(
                            oT_sb[p][64 * h01 : 64 * (h01 + 1), MC * c : MC * (c + 1)],
                            oaug[h01][0:DK, :],
                            bc,
                        )

        # ---- stage 3: output projection ----
        with tc.tile_pool(name="op_ps", bufs=4, space="PSUM") as opps:
            for t in range(NT):
                op = [
                    opps.tile([P, MC], f32, tag="op", name="op_t")
                    for _ in range(E // MC)
                ]
                for p in range(PAIRS):
                    for ec in range(E // MC):
                        nc.tensor.matmul(
                            op[ec],
                            oT_sb[p][:, P * t : P * (t + 1)],
                            wo_sb[:, p, MC * ec : MC * (ec + 1)],
                            start=(p == 0),
                            stop=(p == PAIRS - 1),
                        )
                for ec in range(E // MC):
                    ost = ostpool.tile([P, MC], f32, tag="ost", name="ost_t")
                    nc.vector.tensor_copy(ost, op[ec])
                    nc.sync.dma_start(
                        out=out[P * t : P * (t + 1), MC * ec : MC * (ec + 1)],
                        in_=ost,
                    )

    nc.compile()
    return nc


def _host_inputs(key, value, query, Wk, Wq, Wv, Wo, bq, bk, bv, bias_qk, bias_v):
    """Per-core input maps (host-side shard/transpose/cast — not timed)."""
    tri = np.triu(np.ones((P, P), np.float32)).astype(BF16)  # allowed: n<=m
    in_maps = []
    xT = {}
    for b in range(B):
        xT[("q", b)] = np.ascontiguousarray(query[b].T).astype(BF16)
        xT[("k", b)] = np.ascontiguousarray(key[b].T).astype(BF16)
        xT[("v", b)] = np.ascontiguousarray(value[b].T).astype(BF16)
    for c in range(NCORES):
        b, g = divmod(c, G)
        sl = slice(DKH * g, DKH * (g + 1))
        m = {
            "xqT": xT[("q", b)],
            "xkT": xT[("k", b)],
            "xvT": xT[("v", b)],
            "wqT": np.ascontiguousarray(Wq[sl].T).astype(BF16),
            "wkT": np.ascontiguousarray(Wk[sl].T).astype(BF16),
            "wvT": np.ascontiguousarray(Wv[sl].T).astype(BF16),
            "woT": np.ascontiguousarray(Wo[:, sl].T).astype(BF16),
            "dmask": tri,
        }
        if bias_qk:
            m["bq"] = np.ascontiguousarray(bq[sl].astype(np.float32).reshape(DKH, 1))
            m["bk"] = np.ascontiguousarray(bk[sl].astype(np.float32).reshape(DKH, 1))
        if bias_v:
            m["bv"] = np.ascontiguousarray(bv[sl].astype(np.float32).reshape(1, DKH))
        in_maps.append(m)
    return in_maps


def _numpy_fallback(key, value, query, mask, Wk, bk, Wq, bq, Wv, bv, Wo, bo):
    """Exact reference semantics in numpy (general-mask fallback)."""
    def proj(x, W, b):
        return x @ W.T + b

    k = proj(key, Wk, bk).reshape(B, S, H, DK).transpose(0, 2, 1, 3)
    q = proj(query, Wq, bq).reshape(B, S, H, DK).transpose(0, 2, 1, 3)
    v = proj(value, Wv, bv).reshape(B, S, H, DK).transpose(0, 2, 1, 3)
    scores = np.einsum("bhmd,bhnd->bhmn", q, k).astype(np.float32)
    scores = np.where(mask, scores, np.float32(-1e10)) * np.float32(SCALE)
    scores -= scores.max(axis=3, keepdims=True)
    e = np.exp(scores)
    attn = e / e.sum(axis=3, keepdims=True)
    o = np.einsum("bhmn,bhnv->bhmv", attn, v)
    o = o.transpose(0, 2, 1, 3).reshape(B, S, E)
    return (o @ Wo.T + bo).astype(np.float32)


_program_cache = {}


def kernel(key, value, query, mask, Wk, bk, Wq, bq, Wv, bv, Wo, bo):
    key = np.asarray(key, np.float32)
    value = np.asarray(value, np.float32)
    query = np.asarray(query, np.float32)
    mask = np.asarray(mask)
    Wk, bk = np.asarray(Wk, np.float32), np.asarray(bk, np.float32)
    Wq, bq = np.asarray(Wq, np.float32), np.asarray(bq, np.float32)
    Wv, bv = np.asarray(Wv, np.float32), np.asarray(bv, np.float32)
    Wo, bo = np.asarray(Wo, np.float32), np.asarray(bo, np.float32)

    m2 = mask.reshape(B, S, S) if mask.size == B * S * S else None
    causal = m2 is not None and all(
        np.array_equal(m2[b], np.tril(np.ones((S, S), bool))) for b in range(B)
    )
    allones = m2 is not None and bool(mask.all())
    if not causal and not allones:
        return _numpy_fallback(key, value, query, mask, Wk, bk, Wq, bq, Wv, bv, Wo, bo)

    if causal:
        chunk_ntiles = tuple(4 * (c + 1) for c in range(NMC))
    else:
        chunk_ntiles = tuple(NT for _ in range(NMC))

    bias_qk = bool(np.any(bq) or np.any(bk))
    bias_v = bool(np.any(bv))

    pkey = (chunk_ntiles, causal, bias_qk, bias_v)
    if pkey not in _program_cache:
        _program_cache[pkey] = _build_program(chunk_ntiles, causal, bias_qk, bias_v)
    nc = _program_cache[pkey]

    from concourse.bass_utils import run_bass_kernel_spmd

    in_maps = _host_inputs(key, value, query, Wk, Wq, Wv, Wo, bq, bk, bv, bias_qk, bias_v)
    res = run_bass_kernel_spmd(nc, in_maps, core_ids=list(range(NCORES)))

    outp = np.zeros((B, S, E), np.float32)
    for c in range(NCORES):
        outp[c // G] += res.results[c]["out"]
    outp += bo.astype(np.float32)
    return outp
